# revision 3
# baseline (speedup 1.0000x reference)
"""MoE (top-2 of 8 experts, d=1024) — expert-parallel Bass kernel for 8 trn2 cores.

Strategy (per sharding_hint "Expert-parallel"): shard W1/W2/b1/b2 along the
expert axis (expert e -> core e). The host computes the gate scores and top-2
assignment (0.2% of model FLOPs, deterministic) to build the dispatch: each
core receives exactly the tokens routed to its expert (padded to a fixed
capacity C), transposed to [d, C] so the device needs no transposes. Each core
computes   yT = (relu(W1^T xT + b1)^T W2 + b2) * w   with float32r matmuls
(full PE rate), and the host scatter-adds the two expert contributions per
token (the "combine" of the return all-to-all).
"""

import numpy as np

import concourse.bass as bass
import concourse.mybir as mybir
import concourse.tile as tile
from concourse import bacc
from concourse.bass_utils import run_bass_kernel_spmd

# Problem shapes (hardcoded per contract)
D = 1024  # d_model == d_hidden
N_EXPERTS = 8
TOP_K = 2
N_CORES = 8
B, T = 4, 2048
N_TOKENS = B * T

F32 = mybir.dt.float32
F32R = mybir.dt.float32r
KC = D // 128  # contraction chunks (8)
MC = D // 128  # output-feature chunks (8)
NT = 512      # tokens per matmul (moving free dim; fp32 max)


def build_moe_expert_kernel(C: int, repeat: int = 1) -> bacc.Bacc:
    """One-expert MLP kernel: yT[d, C] = (relu(x@W1+b1)@W2 + b2) * w, transposed.

    DRAM inputs: xT [D, C] (tokens transposed), wvec [1, C] combine weights,
    w1 [D, D], b1 [D], w2 [D, D], b2 [D]. Output: yT [D, C].
    `repeat` re-emits the whole computation (for slope-based HW timing).
    """
    assert C % NT == 0
    nch = C // NT

    nc = bacc.Bacc("TRN2", target_bir_lowering=False, debug=False,
                   num_devices=N_CORES)

    xT = nc.dram_tensor("xT", [D, C], F32R, kind="ExternalInput")
    wvec = nc.dram_tensor("wvec", [1, C], F32, kind="ExternalInput")
    w1 = nc.dram_tensor("w1", [D, D], F32R, kind="ExternalInput")
    b1 = nc.dram_tensor("b1", [D], F32, kind="ExternalInput")
    w2 = nc.dram_tensor("w2", [D, D], F32R, kind="ExternalInput")
    b2 = nc.dram_tensor("b2", [D], F32, kind="ExternalInput")
    yT = nc.dram_tensor("yT", [D, C], F32, kind="ExternalOutput")

    # DRAM views: partition-dim-first tilings
    xT_v = xT.ap().rearrange("(kc kp) c -> kp kc c", kc=KC)    # [128, KC, C]
    w1_v = w1.ap().rearrange("(kc kp) m -> kp kc m", kc=KC)    # [128, KC, D]
    w2_v = w2.ap().rearrange("(kc kp) m -> kp kc m", kc=KC)
    b1_v = b1.ap().rearrange("(mc mp) -> mp mc", mc=MC)        # [128, MC]
    b2_v = b2.ap().rearrange("(mc mp) -> mp mc", mc=MC)
    yT_v = yT.ap().rearrange("(mc mp) c -> mp mc c", mc=MC)    # [128, MC, C]
    # partition-broadcast view of wvec for DMA: [128, C] with partition step 0
    wvec_b = bass.AP(tensor=wvec.ap().tensor, offset=wvec.ap().offset,
                     ap=[[0, 128]] + list(wvec.ap().ap[1:]))

    with tile.TileContext(nc) as tc:
        with (
            tc.tile_pool(name="weights", bufs=1) as wpool,
            tc.tile_pool(name="consts", bufs=1) as cpool,
            tc.tile_pool(name="xin", bufs=2) as xpool,
            tc.tile_pool(name="hmid", bufs=2) as hpool,
            tc.tile_pool(name="yout", bufs=2) as ypool,
            tc.tile_pool(name="ph", bufs=3, space="PSUM") as phpool,
            tc.tile_pool(name="py", bufs=3, space="PSUM") as pypool,
        ):
            for _ in range(repeat):
                w1_sb = wpool.tile([128, KC, D], F32R, tag="w1")
                w2_sb = wpool.tile([128, KC, D], F32R, tag="w2")
                nc.sync.dma_start(w1_sb[:], w1_v)
                nc.sync.dma_start(w2_sb[:], w2_v)
                b1_sb = cpool.tile([128, MC], F32, tag="b1")
                b2_sb = cpool.tile([128, MC], F32, tag="b2")
                nc.sync.dma_start(b1_sb[:], b1_v)
                nc.sync.dma_start(b2_sb[:], b2_v)
                wb_sb = cpool.tile([128, C], F32, tag="wb")
                nc.sync.dma_start(wb_sb[:], wvec_b)

                for n in range(nch):
                    ns = bass.ts(n, NT)
                    x_sb = xpool.tile([128, KC, NT], F32R, tag="x")
                    nc.sync.dma_start(x_sb[:], xT_v[:, :, ns])

                    h_sb = hpool.tile([128, KC, NT], F32R, tag="h")
                    for mc in range(MC):
                        ph = phpool.tile([128, NT], F32, tag="ph")
                        for kc in range(KC):
                            nc.tensor.matmul(
                                ph[:],
                                w1_sb[:, kc, bass.ts(mc, 128)],
                                x_sb[:, kc, :],
                                start=(kc == 0), stop=(kc == KC - 1),
                            )
                        # h = relu(ph + b1)
                        nc.scalar.activation(
                            h_sb[:, mc, :], ph[:],
                            mybir.ActivationFunctionType.Relu,
                            bias=b1_sb[:, mc:mc + 1],
                        )

                    y_sb = ypool.tile([128, MC, NT], F32, tag="y")
                    for mc in range(MC):
                        py = pypool.tile([128, NT], F32, tag="py")
                        for kc in range(KC):
                            nc.tensor.matmul(
                                py[:],
                                w2_sb[:, kc, bass.ts(mc, 128)],
                                h_sb[:, kc, :],
                                start=(kc == 0), stop=(kc == KC - 1),
                            )
                        # y = (py + b2) * w
                        nc.scalar.activation(
                            y_sb[:, mc, :], py[:],
                            mybir.ActivationFunctionType.Identity,
                            bias=b2_sb[:, mc:mc + 1],
                        )
                        nc.vector.tensor_mul(
                            y_sb[:, mc, :], y_sb[:, mc, :], wb_sb[:, ns],
                        )
                    nc.sync.dma_start(yT_v[:, :, ns], y_sb[:])

    nc.compile()
    return nc


_NC_CACHE: dict = {}


def _get_kernel(C: int, repeat: int = 1) -> bacc.Bacc:
    key = (C, repeat)
    if key not in _NC_CACHE:
        _NC_CACHE[key] = build_moe_expert_kernel(C, repeat)
    return _NC_CACHE[key]


def dispatch(x, W_gate, b_gate):
    """Host-side gate + top-2 dispatch plan. Returns (ids, wts, C)."""
    xf = np.ascontiguousarray(x.reshape(-1, D), dtype=np.float32)
    scores = xf @ np.asarray(W_gate, np.float32) + np.asarray(b_gate, np.float32)
    # top-2 expert ids per token (order irrelevant: contributions are summed)
    top2 = np.argpartition(scores, N_EXPERTS - TOP_K, axis=1)[:, -TOP_K:]
    ids, wts = [], []
    for e in range(N_EXPERTS):
        tok = np.nonzero((top2 == e).any(axis=1))[0]
        ids.append(tok)
        wts.append(scores[tok, e])
    max_cnt = max(len(t) for t in ids)
    C = ((max_cnt + NT - 1) // NT) * NT
    return xf, ids, wts, C


def kernel(x, W_gate, b_gate, W1, b1, W2, b2):
    xf, ids, wts, C = dispatch(x, W_gate, b_gate)
    nc = _get_kernel(C)

    W1 = np.asarray(W1, np.float32)
    W2 = np.asarray(W2, np.float32)
    b1 = np.asarray(b1, np.float32)
    b2 = np.asarray(b2, np.float32)

    in_maps = []
    for e in range(N_EXPERTS):
        cnt = len(ids[e])
        xTe = np.zeros((D, C), np.float32)
        xTe[:, :cnt] = xf[ids[e]].T
        wv = np.zeros((1, C), np.float32)
        wv[0, :cnt] = wts[e]
        in_maps.append({
            "xT": xTe, "wvec": wv,
            "w1": np.ascontiguousarray(W1[e]), "b1": b1[e],
            "w2": np.ascontiguousarray(W2[e]), "b2": b2[e],
        })

    res = run_bass_kernel_spmd(nc, in_maps, core_ids=list(range(N_CORES)))

    out = np.zeros((N_TOKENS, D), np.float32)
    for e in range(N_EXPERTS):
        cnt = len(ids[e])
        out[ids[e]] += res.results[e]["yT"].T[:cnt]
    return out.reshape(B, T, D)


# revision 4
# speedup vs baseline: 132.0191x; 132.0191x over previous
"""MoE (top-2 of 8 experts, d=1024) — expert-parallel Bass kernel for 8 trn2 cores.

Strategy (per sharding_hint "Expert-parallel"): shard W1/W2/b1/b2 along the
expert axis (expert e -> core e). The host computes the gate scores and top-2
assignment (0.2% of model FLOPs, deterministic) to build the dispatch: each
core receives exactly the tokens routed to its expert (padded to a fixed
capacity C), transposed to [d, C] so the device needs no transposes. Each core
computes   yT = (relu(W1^T xT + b1)^T W2 + b2) * w   with float32r matmuls
(full PE rate), and the host scatter-adds the two expert contributions per
token (the "combine" of the return all-to-all).
"""

import numpy as np

import concourse.bass as bass
import concourse.mybir as mybir
import concourse.tile as tile
from concourse import bacc
from concourse.bass_utils import run_bass_kernel_spmd

# Problem shapes (hardcoded per contract)
D = 1024  # d_model == d_hidden
N_EXPERTS = 8
TOP_K = 2
N_CORES = 8
B, T = 4, 2048
N_TOKENS = B * T

F32 = mybir.dt.float32
F32R = mybir.dt.float32r
KC = D // 128  # contraction chunks (8)
MC = D // 128  # output-feature chunks (8)
NT = 512      # tokens per matmul (moving free dim; fp32 max)


def build_moe_expert_kernel(C: int, repeat: int = 1) -> bacc.Bacc:
    """One-expert MLP kernel: yT[d, C] = (relu(x@W1+b1)@W2 + b2) * w, transposed.

    DRAM inputs: xT [D, C] (tokens transposed), wvec [1, C] combine weights,
    w1 [D, D], b1 [D], w2 [D, D], b2 [D]. Output: yT [D, C].
    `repeat` re-emits the whole computation (for slope-based HW timing).
    """
    assert C % NT == 0
    nch = C // NT

    nc = bacc.Bacc("TRN2", target_bir_lowering=False, debug=False,
                   num_devices=N_CORES)

    xT = nc.dram_tensor("xT", [D, C], F32R, kind="ExternalInput")
    wvec = nc.dram_tensor("wvec", [1, C], F32, kind="ExternalInput")
    w1 = nc.dram_tensor("w1", [D, D], F32R, kind="ExternalInput")
    b1 = nc.dram_tensor("b1", [D], F32, kind="ExternalInput")
    w2 = nc.dram_tensor("w2", [D, D], F32R, kind="ExternalInput")
    b2 = nc.dram_tensor("b2", [D], F32, kind="ExternalInput")
    yT = nc.dram_tensor("yT", [D, C], F32, kind="ExternalOutput")

    # DRAM views: partition-dim-first tilings
    xT_v = xT.ap().rearrange("(kc kp) c -> kp kc c", kc=KC)    # [128, KC, C]
    w1_v = w1.ap().rearrange("(kc kp) m -> kp kc m", kc=KC)    # [128, KC, D]
    w2_v = w2.ap().rearrange("(kc kp) m -> kp kc m", kc=KC)
    b1_v = b1.ap().rearrange("(mc mp) -> mp mc", mc=MC)        # [128, MC]
    b2_v = b2.ap().rearrange("(mc mp) -> mp mc", mc=MC)
    yT_v = yT.ap().rearrange("(mc mp) c -> mp mc c", mc=MC)    # [128, MC, C]
    # partition-broadcast view of wvec for DMA: [128, C] with partition step 0
    wvec_b = bass.AP(tensor=wvec.ap().tensor, offset=wvec.ap().offset,
                     ap=[[0, 128]] + list(wvec.ap().ap[1:]))

    with tile.TileContext(nc) as tc:
        with (
            tc.tile_pool(name="weights", bufs=1) as wpool,
            tc.tile_pool(name="consts", bufs=1) as cpool,
            tc.tile_pool(name="xin", bufs=2) as xpool,
            tc.tile_pool(name="hmid", bufs=2) as hpool,
            tc.tile_pool(name="yout", bufs=2) as ypool,
            tc.tile_pool(name="ph", bufs=3, space="PSUM") as phpool,
            tc.tile_pool(name="py", bufs=3, space="PSUM") as pypool,
        ):
            from contextlib import nullcontext
            loop_cm = (
                tc.For_i(0, repeat, 1,
                         hint_engines=(mybir.EngineType.PE,
                                       mybir.EngineType.Activation,
                                       mybir.EngineType.DVE,
                                       mybir.EngineType.SP))
                if repeat > 1 else nullcontext()
            )
            with loop_cm:
                w1_sb = wpool.tile([128, KC, D], F32R, tag="w1")
                w2_sb = wpool.tile([128, KC, D], F32R, tag="w2")
                nc.sync.dma_start(w1_sb[:], w1_v)
                nc.sync.dma_start(w2_sb[:], w2_v)
                b1_sb = cpool.tile([128, MC], F32, tag="b1")
                b2_sb = cpool.tile([128, MC], F32, tag="b2")
                nc.sync.dma_start(b1_sb[:], b1_v)
                nc.sync.dma_start(b2_sb[:], b2_v)
                wb_sb = cpool.tile([128, C], F32, tag="wb")
                nc.sync.dma_start(wb_sb[:], wvec_b)

                for n in range(nch):
                    ns = bass.ts(n, NT)
                    x_sb = xpool.tile([128, KC, NT], F32R, tag="x")
                    nc.sync.dma_start(x_sb[:], xT_v[:, :, ns])

                    h_sb = hpool.tile([128, KC, NT], F32R, tag="h")
                    for mc in range(MC):
                        ph = phpool.tile([128, NT], F32, tag="ph")
                        for kc in range(KC):
                            nc.tensor.matmul(
                                ph[:],
                                w1_sb[:, kc, bass.ts(mc, 128)],
                                x_sb[:, kc, :],
                                start=(kc == 0), stop=(kc == KC - 1),
                            )
                        # h = relu(ph + b1)
                        nc.scalar.activation(
                            h_sb[:, mc, :], ph[:],
                            mybir.ActivationFunctionType.Relu,
                            bias=b1_sb[:, mc:mc + 1],
                        )

                    y_sb = ypool.tile([128, MC, NT], F32, tag="y")
                    for mc in range(MC):
                        py = pypool.tile([128, NT], F32, tag="py")
                        for kc in range(KC):
                            nc.tensor.matmul(
                                py[:],
                                w2_sb[:, kc, bass.ts(mc, 128)],
                                h_sb[:, kc, :],
                                start=(kc == 0), stop=(kc == KC - 1),
                            )
                        # y = (py + b2) * w
                        nc.scalar.activation(
                            y_sb[:, mc, :], py[:],
                            mybir.ActivationFunctionType.Identity,
                            bias=b2_sb[:, mc:mc + 1],
                        )
                        nc.vector.tensor_mul(
                            y_sb[:, mc, :], y_sb[:, mc, :], wb_sb[:, ns],
                        )
                    nc.sync.dma_start(yT_v[:, :, ns], y_sb[:])

    nc.compile()
    return nc


_NC_CACHE: dict = {}


def _get_kernel(C: int, repeat: int = 1) -> bacc.Bacc:
    key = (C, repeat)
    if key not in _NC_CACHE:
        _NC_CACHE[key] = build_moe_expert_kernel(C, repeat)
    return _NC_CACHE[key]


def dispatch(x, W_gate, b_gate):
    """Host-side gate + top-2 dispatch plan. Returns (ids, wts, C)."""
    xf = np.ascontiguousarray(x.reshape(-1, D), dtype=np.float32)
    scores = xf @ np.asarray(W_gate, np.float32) + np.asarray(b_gate, np.float32)
    # top-2 expert ids per token (order irrelevant: contributions are summed)
    top2 = np.argpartition(scores, N_EXPERTS - TOP_K, axis=1)[:, -TOP_K:]
    ids, wts = [], []
    for e in range(N_EXPERTS):
        tok = np.nonzero((top2 == e).any(axis=1))[0]
        ids.append(tok)
        wts.append(scores[tok, e])
    max_cnt = max(len(t) for t in ids)
    C = ((max_cnt + NT - 1) // NT) * NT
    return xf, ids, wts, C


def kernel(x, W_gate, b_gate, W1, b1, W2, b2):
    xf, ids, wts, C = dispatch(x, W_gate, b_gate)
    nc = _get_kernel(C)

    W1 = np.asarray(W1, np.float32)
    W2 = np.asarray(W2, np.float32)
    b1 = np.asarray(b1, np.float32)
    b2 = np.asarray(b2, np.float32)

    in_maps = []
    for e in range(N_EXPERTS):
        cnt = len(ids[e])
        xTe = np.zeros((D, C), np.float32)
        xTe[:, :cnt] = xf[ids[e]].T
        wv = np.zeros((1, C), np.float32)
        wv[0, :cnt] = wts[e]
        in_maps.append({
            "xT": xTe, "wvec": wv,
            "w1": np.ascontiguousarray(W1[e]), "b1": b1[e],
            "w2": np.ascontiguousarray(W2[e]), "b2": b2[e],
        })

    res = run_bass_kernel_spmd(nc, in_maps, core_ids=list(range(N_CORES)))

    out = np.zeros((N_TOKENS, D), np.float32)
    for e in range(N_EXPERTS):
        cnt = len(ids[e])
        out[ids[e]] += res.results[e]["yT"].T[:cnt]
    return out.reshape(B, T, D)


# revision 5
# speedup vs baseline: 140.2236x; 1.0621x over previous
"""MoE (top-2 of 8 experts, d=1024) — expert-parallel Bass kernel for 8 trn2 cores.

Strategy (per sharding_hint "Expert-parallel"): shard W1/W2/b1/b2 along the
expert axis (expert e -> core e). The host computes the gate scores and top-2
assignment (0.2% of model FLOPs, deterministic) to build the dispatch: each
core receives exactly the tokens routed to its expert (padded to a fixed
capacity C), transposed and chunk-major ([nch, d, 512] blocks, each fully
contiguous for sequential HBM streaming). Each core computes
   yT = (relu(W1^T xT + b1)^T W2 + b2) * w
with float32r matmuls (full PE rate, ~1e-4 accuracy), and the host
scatter-adds the two expert contributions per token (the "combine" of the
return all-to-all).
"""

import numpy as np

import concourse.bass as bass
import concourse.mybir as mybir
import concourse.tile as tile
from concourse import bacc
from concourse.bass_utils import run_bass_kernel_spmd

# Problem shapes (hardcoded per contract)
D = 1024  # d_model == d_hidden
N_EXPERTS = 8
TOP_K = 2
N_CORES = 8
B, T = 4, 2048
N_TOKENS = B * T

F32 = mybir.dt.float32
F32R = mybir.dt.float32r
KC = D // 128  # contraction chunks (8)
MC = D // 128  # output-feature chunks (8)
NT = 512      # tokens per matmul (moving free dim; fp32 max)


def build_moe_expert_kernel(C: int, repeat: int = 1) -> bacc.Bacc:
    """One-expert MLP kernel: yT = (relu(x@W1+b1)@W2 + b2) * w, chunk-major.

    DRAM inputs: xT [nch, D, NT] (tokens transposed, chunk-major), wvec [1, C]
    combine weights, w1 [D, D], b1 [D], w2 [D, D], b2 [D].
    Output: yT [nch, D, NT].
    `repeat` wraps the computation in a hardware loop (slope-based HW timing).
    """
    assert C % NT == 0
    nch = C // NT

    nc = bacc.Bacc("TRN2", target_bir_lowering=False, debug=False,
                   num_devices=N_CORES)

    xT = nc.dram_tensor("xT", [nch, D, NT], F32R, kind="ExternalInput")
    wvec = nc.dram_tensor("wvec", [1, C], F32, kind="ExternalInput")
    w1 = nc.dram_tensor("w1", [D, D], F32R, kind="ExternalInput")
    b1 = nc.dram_tensor("b1", [D], F32, kind="ExternalInput")
    w2 = nc.dram_tensor("w2", [D, D], F32R, kind="ExternalInput")
    b2 = nc.dram_tensor("b2", [D], F32, kind="ExternalInput")
    yT = nc.dram_tensor("yT", [nch, D, NT], F32, kind="ExternalOutput")

    # DRAM views: partition-dim-first tilings (chunk blocks are contiguous)
    xT_v = xT.ap().rearrange("n (kc kp) t -> n kp kc t", kc=KC)  # [nch,128,KC,NT]
    w1_v = w1.ap().rearrange("(kc kp) m -> kp kc m", kc=KC)      # [128, KC, D]
    w2_v = w2.ap().rearrange("(kc kp) m -> kp kc m", kc=KC)
    b1_v = b1.ap().rearrange("(mc mp) -> mp mc", mc=MC)          # [128, MC]
    b2_v = b2.ap().rearrange("(mc mp) -> mp mc", mc=MC)
    yT_v = yT.ap().rearrange("n (mc mp) t -> n mp mc t", mc=MC)  # [nch,128,MC,NT]
    # partition-broadcast view of wvec for DMA: [128, C] with partition step 0
    wvec_b = bass.AP(tensor=wvec.ap().tensor, offset=wvec.ap().offset,
                     ap=[[0, 128]] + list(wvec.ap().ap[1:]))

    with tile.TileContext(nc) as tc:
        with (
            tc.tile_pool(name="weights", bufs=1) as wpool,
            tc.tile_pool(name="consts", bufs=1) as cpool,
            tc.tile_pool(name="xin", bufs=3) as xpool,
            tc.tile_pool(name="hmid", bufs=2) as hpool,
            tc.tile_pool(name="yout", bufs=2) as ypool,
            tc.tile_pool(name="ph", bufs=3, space="PSUM") as phpool,
            tc.tile_pool(name="py", bufs=3, space="PSUM") as pypool,
        ):
            from contextlib import nullcontext
            loop_cm = (
                tc.For_i(0, repeat, 1,
                         hint_engines=(mybir.EngineType.PE,
                                       mybir.EngineType.Activation,
                                       mybir.EngineType.DVE,
                                       mybir.EngineType.SP))
                if repeat > 1 else nullcontext()
            )
            with loop_cm:
                w1_sb = wpool.tile([128, KC, D], F32R, tag="w1")
                w2_sb = wpool.tile([128, KC, D], F32R, tag="w2")
                nc.sync.dma_start(w1_sb[:], w1_v)
                nc.sync.dma_start(w2_sb[:], w2_v)
                b1_sb = cpool.tile([128, MC], F32, tag="b1")
                b2_sb = cpool.tile([128, MC], F32, tag="b2")
                nc.sync.dma_start(b1_sb[:], b1_v)
                nc.sync.dma_start(b2_sb[:], b2_v)
                wb_sb = cpool.tile([128, C], F32, tag="wb")
                nc.sync.dma_start(wb_sb[:], wvec_b)

                for n in range(nch):
                    ns = bass.ts(n, NT)
                    x_sb = xpool.tile([128, KC, NT], F32R, tag="x")
                    nc.sync.dma_start(x_sb[:], xT_v[n])

                    h_sb = hpool.tile([128, KC, NT], F32R, tag="h")
                    for mc in range(MC):
                        ph = phpool.tile([128, NT], F32, tag="ph")
                        for kc in range(KC):
                            nc.tensor.matmul(
                                ph[:],
                                w1_sb[:, kc, bass.ts(mc, 128)],
                                x_sb[:, kc, :],
                                start=(kc == 0), stop=(kc == KC - 1),
                            )
                        # h = relu(ph + b1)
                        nc.scalar.activation(
                            h_sb[:, mc, :], ph[:],
                            mybir.ActivationFunctionType.Relu,
                            bias=b1_sb[:, mc:mc + 1],
                        )

                    y_sb = ypool.tile([128, MC, NT], F32, tag="y")
                    for mc in range(MC):
                        py = pypool.tile([128, NT], F32, tag="py")
                        for kc in range(KC):
                            nc.tensor.matmul(
                                py[:],
                                w2_sb[:, kc, bass.ts(mc, 128)],
                                h_sb[:, kc, :],
                                start=(kc == 0), stop=(kc == KC - 1),
                            )
                        # y = (py + b2) * w
                        nc.scalar.activation(
                            y_sb[:, mc, :], py[:],
                            mybir.ActivationFunctionType.Identity,
                            bias=b2_sb[:, mc:mc + 1],
                        )
                        nc.vector.tensor_mul(
                            y_sb[:, mc, :], y_sb[:, mc, :], wb_sb[:, ns],
                        )
                    nc.sync.dma_start(yT_v[n], y_sb[:])

    nc.compile()
    return nc


_NC_CACHE: dict = {}


def _get_kernel(C: int, repeat: int = 1) -> bacc.Bacc:
    key = (C, repeat)
    if key not in _NC_CACHE:
        _NC_CACHE[key] = build_moe_expert_kernel(C, repeat)
    return _NC_CACHE[key]


def dispatch(x, W_gate, b_gate):
    """Host-side gate + top-2 dispatch plan. Returns (xf, ids, wts, C)."""
    xf = np.ascontiguousarray(np.asarray(x).reshape(-1, D), dtype=np.float32)
    scores = xf @ np.asarray(W_gate, np.float32) + np.asarray(b_gate, np.float32)
    # top-2 expert ids per token (order irrelevant: contributions are summed)
    top2 = np.argpartition(scores, N_EXPERTS - TOP_K, axis=1)[:, -TOP_K:]
    ids, wts = [], []
    for e in range(N_EXPERTS):
        tok = np.nonzero((top2 == e).any(axis=1))[0]
        ids.append(tok)
        wts.append(scores[tok, e])
    max_cnt = max(len(t) for t in ids)
    C = ((max_cnt + NT - 1) // NT) * NT
    return xf, ids, wts, C


def make_in_maps(inputs_or_parts, xf, ids, wts, C):
    """Build per-core input dicts (chunk-major xT blocks)."""
    W1, b1, W2, b2 = inputs_or_parts
    nch = C // NT
    in_maps = []
    for e in range(N_EXPERTS):
        cnt = len(ids[e])
        xTe = np.zeros((D, C), np.float32)
        xTe[:, :cnt] = xf[ids[e]].T
        xTe = np.ascontiguousarray(
            xTe.reshape(D, nch, NT).transpose(1, 0, 2))  # [nch, D, NT]
        wv = np.zeros((1, C), np.float32)
        wv[0, :cnt] = wts[e]
        in_maps.append({
            "xT": xTe, "wvec": wv,
            "w1": np.ascontiguousarray(W1[e]), "b1": b1[e],
            "w2": np.ascontiguousarray(W2[e]), "b2": b2[e],
        })
    return in_maps


def kernel(x, W_gate, b_gate, W1, b1, W2, b2):
    xf, ids, wts, C = dispatch(x, W_gate, b_gate)
    nc = _get_kernel(C)

    W1 = np.asarray(W1, np.float32)
    W2 = np.asarray(W2, np.float32)
    b1 = np.asarray(b1, np.float32)
    b2 = np.asarray(b2, np.float32)
    in_maps = make_in_maps((W1, b1, W2, b2), xf, ids, wts, C)

    res = run_bass_kernel_spmd(nc, in_maps, core_ids=list(range(N_CORES)))

    out = np.zeros((N_TOKENS, D), np.float32)
    for e in range(N_EXPERTS):
        cnt = len(ids[e])
        yTe = res.results[e]["yT"]            # [nch, D, NT]
        yTe = yTe.transpose(1, 0, 2).reshape(D, C)
        out[ids[e]] += yTe.T[:cnt]
    return out.reshape(B, T, D)


# revision 7
# speedup vs baseline: 140.6915x; 1.0033x over previous
"""MoE (top-2 of 8 experts, d=1024) — expert-parallel Bass kernel for 8 trn2 cores.

Strategy (per sharding_hint "Expert-parallel"): shard W1/W2/b1/b2 along the
expert axis (expert e -> core e). The host computes the gate scores and top-2
assignment (0.2% of model FLOPs, deterministic) to build the dispatch: each
core receives exactly the tokens routed to its expert (padded to a fixed
capacity C), transposed and chunk-major ([nch, d, 512] blocks, each fully
contiguous for sequential HBM streaming). Each core computes
   yT = (relu(W1^T xT + b1)^T W2 + b2) * w
with float32r matmuls (full PE rate, ~1e-4 accuracy), and the host
scatter-adds the two expert contributions per token (the "combine" of the
return all-to-all).
"""

import numpy as np

import concourse.bass as bass
import concourse.mybir as mybir
import concourse.tile as tile
from concourse import bacc
from concourse.bass_utils import run_bass_kernel_spmd

# Problem shapes (hardcoded per contract)
D = 1024  # d_model == d_hidden
N_EXPERTS = 8
TOP_K = 2
N_CORES = 8
B, T = 4, 2048
N_TOKENS = B * T

F32 = mybir.dt.float32
F32R = mybir.dt.float32r
KC = D // 128  # contraction chunks (8)
MC = D // 128  # output-feature chunks (8)
NT = 512      # tokens per matmul (moving free dim; fp32 max)


def build_moe_expert_kernel(C: int, repeat: int = 1) -> bacc.Bacc:
    """One-expert MLP kernel: yT = (relu(x@W1+b1)@W2 + b2) * w, chunk-major.

    DRAM inputs: xT [nch, D, NT] (tokens transposed, chunk-major), wvec [1, C]
    combine weights, w1 [D, D], b1 [D], w2 [D, D], b2 [D].
    Output: yT [nch, D, NT].
    `repeat` wraps the computation in a hardware loop (slope-based HW timing).
    """
    assert C % NT == 0
    nch = C // NT

    nc = bacc.Bacc("TRN2", target_bir_lowering=False, debug=False,
                   num_devices=N_CORES)

    xT = nc.dram_tensor("xT", [nch, D, NT], F32R, kind="ExternalInput")
    wvec = nc.dram_tensor("wvec", [1, C], F32, kind="ExternalInput")
    w1 = nc.dram_tensor("w1", [D, D], F32R, kind="ExternalInput")
    b1 = nc.dram_tensor("b1", [D], F32, kind="ExternalInput")
    w2 = nc.dram_tensor("w2", [D, D], F32R, kind="ExternalInput")
    b2 = nc.dram_tensor("b2", [D], F32, kind="ExternalInput")
    yT = nc.dram_tensor("yT", [nch, D, NT], F32, kind="ExternalOutput")

    # DRAM views: partition-dim-first tilings (chunk blocks are contiguous)
    xT_v = xT.ap().rearrange("n (kc kp) t -> n kp kc t", kc=KC)  # [nch,128,KC,NT]
    w1_v = w1.ap().rearrange("(kc kp) m -> kp kc m", kc=KC)      # [128, KC, D]
    w2_v = w2.ap().rearrange("(kc kp) m -> kp kc m", kc=KC)
    b1_v = b1.ap().rearrange("(mc mp) -> mp mc", mc=MC)          # [128, MC]
    b2_v = b2.ap().rearrange("(mc mp) -> mp mc", mc=MC)
    yT_v = yT.ap().rearrange("n (mc mp) t -> n mp mc t", mc=MC)  # [nch,128,MC,NT]
    # partition-broadcast view of wvec for DMA: [128, C] with partition step 0
    wvec_b = bass.AP(tensor=wvec.ap().tensor, offset=wvec.ap().offset,
                     ap=[[0, 128]] + list(wvec.ap().ap[1:]))

    with tile.TileContext(nc) as tc:
        with (
            tc.tile_pool(name="weights", bufs=1) as wpool,
            tc.tile_pool(name="consts", bufs=1) as cpool,
            tc.tile_pool(name="xin", bufs=3) as xpool,
            tc.tile_pool(name="hmid", bufs=2) as hpool,
            tc.tile_pool(name="yout", bufs=2) as ypool,
            tc.tile_pool(name="ph", bufs=3, space="PSUM") as phpool,
            tc.tile_pool(name="py", bufs=3, space="PSUM") as pypool,
        ):
            from contextlib import nullcontext
            loop_cm = (
                tc.For_i(0, repeat, 1,
                         hint_engines=(mybir.EngineType.PE,
                                       mybir.EngineType.Activation,
                                       mybir.EngineType.DVE,
                                       mybir.EngineType.SP))
                if repeat > 1 else nullcontext()
            )
            with loop_cm:
                # Per-kc split DMAs: the first matmul only waits for its own
                # 512KB weight slice + 256KB x slice instead of the whole 10MB
                # prologue (model: first MM 36.6us -> 5.1us).
                w1_sb = wpool.tile([128, KC, D], F32R, tag="w1")
                w2_sb = wpool.tile([128, KC, D], F32R, tag="w2")
                b1_sb = cpool.tile([128, MC], F32, tag="b1")
                b2_sb = cpool.tile([128, MC], F32, tag="b2")
                wb_sb = cpool.tile([128, C], F32, tag="wb")
                x0 = xpool.tile([128, KC, NT], F32R, tag="x")
                for kc in range(KC):
                    nc.sync.dma_start(w1_sb[:, kc, :], w1_v[:, kc, :])
                    nc.sync.dma_start(x0[:, kc, :], xT_v[0][:, kc, :])
                nc.sync.dma_start(b1_sb[:], b1_v)
                for kc in range(KC):
                    nc.sync.dma_start(w2_sb[:, kc, :], w2_v[:, kc, :])
                nc.sync.dma_start(b2_sb[:], b2_v)
                nc.sync.dma_start(wb_sb[:], wvec_b)

                for n in range(nch):
                    ns = bass.ts(n, NT)
                    if n == 0:
                        x_sb = x0
                    else:
                        x_sb = xpool.tile([128, KC, NT], F32R, tag="x")
                        for kc in range(KC):
                            nc.sync.dma_start(x_sb[:, kc, :], xT_v[n][:, kc, :])

                    h_sb = hpool.tile([128, KC, NT], F32R, tag="h")
                    for mc in range(MC):
                        ph = phpool.tile([128, NT], F32, tag="ph")
                        for kc in range(KC):
                            nc.tensor.matmul(
                                ph[:],
                                w1_sb[:, kc, bass.ts(mc, 128)],
                                x_sb[:, kc, :],
                                start=(kc == 0), stop=(kc == KC - 1),
                            )
                        # h = relu(ph + b1)
                        nc.scalar.activation(
                            h_sb[:, mc, :], ph[:],
                            mybir.ActivationFunctionType.Relu,
                            bias=b1_sb[:, mc:mc + 1],
                        )

                    y_sb = ypool.tile([128, MC, NT], F32, tag="y")
                    for mc in range(MC):
                        py = pypool.tile([128, NT], F32, tag="py")
                        for kc in range(KC):
                            nc.tensor.matmul(
                                py[:],
                                w2_sb[:, kc, bass.ts(mc, 128)],
                                h_sb[:, kc, :],
                                start=(kc == 0), stop=(kc == KC - 1),
                            )
                        # y = (py + b2) * w
                        nc.scalar.activation(
                            y_sb[:, mc, :], py[:],
                            mybir.ActivationFunctionType.Identity,
                            bias=b2_sb[:, mc:mc + 1],
                        )
                        nc.vector.tensor_mul(
                            y_sb[:, mc, :], y_sb[:, mc, :], wb_sb[:, ns],
                        )
                        nc.sync.dma_start(yT_v[n][:, mc, :], y_sb[:, mc, :])

    nc.compile()
    return nc


_NC_CACHE: dict = {}


def _get_kernel(C: int, repeat: int = 1) -> bacc.Bacc:
    key = (C, repeat)
    if key not in _NC_CACHE:
        _NC_CACHE[key] = build_moe_expert_kernel(C, repeat)
    return _NC_CACHE[key]


def dispatch(x, W_gate, b_gate):
    """Host-side gate + top-2 dispatch plan. Returns (xf, ids, wts, C)."""
    xf = np.ascontiguousarray(np.asarray(x).reshape(-1, D), dtype=np.float32)
    scores = xf @ np.asarray(W_gate, np.float32) + np.asarray(b_gate, np.float32)
    # top-2 expert ids per token (order irrelevant: contributions are summed)
    top2 = np.argpartition(scores, N_EXPERTS - TOP_K, axis=1)[:, -TOP_K:]
    ids, wts = [], []
    for e in range(N_EXPERTS):
        tok = np.nonzero((top2 == e).any(axis=1))[0]
        ids.append(tok)
        wts.append(scores[tok, e])
    max_cnt = max(len(t) for t in ids)
    C = ((max_cnt + NT - 1) // NT) * NT
    return xf, ids, wts, C


def make_in_maps(inputs_or_parts, xf, ids, wts, C):
    """Build per-core input dicts (chunk-major xT blocks)."""
    W1, b1, W2, b2 = inputs_or_parts
    nch = C // NT
    in_maps = []
    for e in range(N_EXPERTS):
        cnt = len(ids[e])
        xTe = np.zeros((D, C), np.float32)
        xTe[:, :cnt] = xf[ids[e]].T
        xTe = np.ascontiguousarray(
            xTe.reshape(D, nch, NT).transpose(1, 0, 2))  # [nch, D, NT]
        wv = np.zeros((1, C), np.float32)
        wv[0, :cnt] = wts[e]
        in_maps.append({
            "xT": xTe, "wvec": wv,
            "w1": np.ascontiguousarray(W1[e]), "b1": b1[e],
            "w2": np.ascontiguousarray(W2[e]), "b2": b2[e],
        })
    return in_maps


def kernel(x, W_gate, b_gate, W1, b1, W2, b2):
    xf, ids, wts, C = dispatch(x, W_gate, b_gate)
    nc = _get_kernel(C)

    W1 = np.asarray(W1, np.float32)
    W2 = np.asarray(W2, np.float32)
    b1 = np.asarray(b1, np.float32)
    b2 = np.asarray(b2, np.float32)
    in_maps = make_in_maps((W1, b1, W2, b2), xf, ids, wts, C)

    res = run_bass_kernel_spmd(nc, in_maps, core_ids=list(range(N_CORES)))

    out = np.zeros((N_TOKENS, D), np.float32)
    for e in range(N_EXPERTS):
        cnt = len(ids[e])
        yTe = res.results[e]["yT"]            # [nch, D, NT]
        yTe = yTe.transpose(1, 0, 2).reshape(D, C)
        out[ids[e]] += yTe.T[:cnt]
    return out.reshape(B, T, D)
